# revision 1
# baseline (speedup 1.0000x reference)
"""Trainium2 Bass kernel for StyleGAN2-style upsampling ConvLayer.

Reference computation (per image):
  zz = conv_transpose2d(x, (w*WSCALE), stride=2)      # 512ch 64x64 -> 256ch 129x129
  y  = upfirdn2d(zz, fir([1,3,3,1]), pad=1, gain=4)   # 4x4 blur   -> 128x128
  y  = clamp(lrelu(y + bias, 0.2) * sqrt(2), +-256)

This implementation keeps the FIR *out* of the tensor engine (the previous
version folded the horizontal FIR into the weights, doubling PE work):

  - PE computes the bare polyphase conv_transpose zz (129x129) in bf16:
    4 parity groups per 16-row block, psum tiles [128, 8, 64], N=512
    matmuls at 1 cyc/row.  Edge strips (row 128 / col 128) are packed
    into one extra psum pass per unit.
  - ACT evacuates PSUM -> SBUF with bf16 downcast and column
    de-interleave into a padded zz plane (row pitch 132 keeps every
    row-shifted view 4B-aligned).
  - DVE applies the vertical FIR [1,3,3,1] as three box passes in bf16
    2x mode, then h1 and part of h3 at 1x.
  - GPSIMD does h2 and the rest of h3.
  - ACT applies Prelu with the FIR normalization (1/16) and lrelu gain
    folded into scale/bias.  Output is bf16; the host upcasts to fp32
    and applies the (numerically inert) +-256 clamp exactly.

Sharding: data parallel, 2 images per core across 8 NeuronCores.
"""

import numpy as np
import ml_dtypes

N_CORES = 8
IMG_PER_CORE = 2
IN_CH, OUT_CH, K, UP = 512, 256, 3, 2
H = W = 64
WSCALE = float(1.0 / np.sqrt(K * K * IN_CH))
ACT_GAIN = float(np.sqrt(2.0))
CLAMP = 256.0
ALPHA = 0.2
N_ICC = IN_CH // 128   # 4 ic chunks
N_OCC = OUT_CH // 128  # 2 oc chunks
N_B = 8                # main blocks of 16 zz rows (rows 0..127)
PW = 132               # padded row width (even -> 264B pitch, keeps bf16 2x)
RING = 32              # v2 ring rows
H_DVE_FRAC = 8.0 / 16.0  # fraction of h-chain rows on DVE (rest gpsimd)

_CACHE = {}
_ABLATE = set()  # debug: subsets of {"stages", "evac", "finish", "edges"}


def _prep_inputs(x, weight, bias):
    bf = ml_dtypes.bfloat16
    n = x.shape[0]
    xq = x.reshape(n, N_ICC, 128, H, W)
    xpad = np.zeros((n, N_ICC, 128, H + 2, W + 2), bf)
    xpad[:, :, :, 1:H + 1, 1:W + 1] = xq.astype(bf)
    # wt[i, (oa, icc, a, b), o]
    w = (weight.astype(np.float64) * WSCALE).astype(np.float32)
    w = w.reshape(N_OCC, 128, N_ICC, 128, K, K)          # [oa, o, icc, i, a, b]
    wt = np.ascontiguousarray(
        w.transpose(3, 0, 2, 4, 5, 1).astype(bf)         # [i, oa, icc, a, b, o]
    ).reshape(128, N_OCC * N_ICC * K * K * 128)
    bg = np.ascontiguousarray(
        (bias.astype(np.float64) * ACT_GAIN).astype(np.float32)
        .reshape(N_OCC, 128).T)
    return xpad, wt, bg


def _build_nc(n_img: int = IMG_PER_CORE, n_rep: int = 1):
    import concourse.bacc as bacc
    import concourse.mybir as mybir
    import concourse.tile as tile

    f32 = mybir.dt.float32
    bf16 = mybir.dt.bfloat16
    Prelu = mybir.ActivationFunctionType.Prelu
    Copy = mybir.ActivationFunctionType.Copy
    Add = mybir.AluOpType.add

    nc = bacc.Bacc()
    xq_ext = nc.declare_dram_parameter(
        "xq", [n_img, N_ICC, 128, H + 2, W + 2], bf16, isOutput=False)
    wt_ext = nc.declare_dram_parameter(
        "wt", [128, N_OCC * N_ICC * K * K * 128], bf16, isOutput=False)
    bg_ext = nc.declare_dram_parameter("bg", [128, N_OCC], f32, isOutput=False)
    out_ext = nc.declare_dram_parameter(
        "out", [n_img, OUT_CH, 2 * H, 2 * W], bf16, isOutput=True)

    def widx(oa, icc, a, b):
        return ((oa * N_ICC + icc) * K + a) * K + b

    with tile.TileContext(nc) as tc:
        with (
            tc.tile_pool(name="cpool", bufs=1) as cpool,
            tc.tile_pool(name="xpool", bufs=2) as xpool,
            tc.tile_pool(name="plane", bufs=1) as plane,
            tc.tile_pool(name="bpool", bufs=2) as bpool,
            tc.tile_pool(name="ppool", bufs=2, space="PSUM") as ppool,
        ):
            # weights ride the ACT ring in parallel with x loads on SP;
            # split per oa so the first matmul gates on half the bytes
            wt = cpool.tile([128, N_OCC * N_ICC * K * K * 128], bf16)
            half = N_ICC * K * K * 128
            nc.scalar.dma_start(out=wt[:, 0:half], in_=wt_ext[:, 0:half])
            nc.scalar.dma_start(out=wt[:, half:2 * half],
                                in_=wt_ext[:, half:2 * half])
            bg = cpool.tile([128, N_OCC], f32)
            nc.scalar.dma_start(out=bg[:], in_=bg_ext[:])

            # persistent planes; sub-range deps give cross-unit pipelining
            zzP = plane.tile([128, PW, PW], bf16)    # zz row r -> slot r+1
            v1P = plane.tile([128, 130, PW], bf16)   # v1[r]=zz[r]+zz[r+1], slot r+1
            v2R = plane.tile([128, RING, PW], bf16)  # v2[r]=v1[r]+v1[r+1], slot (r+1)%RING
            # interleaved view of zzP: [p, rowpar, colpar, r, c]
            # row slot 2r+i, col slot 2c+j
            zzV = zzP[:].rearrange("p (r i) (c j) -> p i j r c", i=2, j=2)

            def lhs(oa, icc, a, b):
                i = widx(oa, icc, a, b)
                return wt[:, i * 128:(i + 1) * 128]

            def unit(img, oa, xts, out_img):
                # ---- padding memsets (pads stay zero through v passes) ----
                nc.vector.memset(zzP[:, 0:1, :], 0.0)
                nc.vector.memset(zzP[:, 130:132, :], 0.0)
                nc.vector.memset(zzP[:, 1:130, 0:1], 0.0)
                nc.vector.memset(zzP[:, 1:130, 130:132], 0.0)

                # ---- edge strips: col 128 (all rows), row 128 (cols 0..127) ----
                # scheduled after block 1 so startup PE work streams in
                # per-icc instead of stalling on the full x load
                def edge_pass():
                    pse = ppool.tile([128, 8, 64], f32, tag="pee")
                    pe_flat = pse[:].rearrange("p r c -> p (r c)")
                    # strip_e: zz[2k,128] k=0..64 -> [0:65]
                    j = 0
                    for icc in range(N_ICC):
                        for al in (0, 1):
                            for be in (0, 1):
                                nc.tensor.matmul(
                                    pe_flat[:, 0:65], lhs(oa, icc, 2 * al, 2 * be),
                                    xts[icc][:, 1 - al:66 - al, 65 - be:66 - be],
                                    start=(j == 0), stop=(j == 15),
                                    skip_group_check=True)
                                j += 1
                    # strip_o: zz[2k+1,128] k=0..63 -> [65:129]
                    j = 0
                    for icc in range(N_ICC):
                        for be in (0, 1):
                            nc.tensor.matmul(
                                pe_flat[:, 65:129], lhs(oa, icc, 1, 2 * be),
                                xts[icc][:, 1:65, 65 - be:66 - be],
                                start=(j == 0), stop=(j == 7),
                                skip_group_check=True)
                            j += 1
                    # R_e: zz[128, 2m] m=0..63 -> [129:193]
                    j = 0
                    for icc in range(N_ICC):
                        for al in (0, 1):
                            for be in (0, 1):
                                nc.tensor.matmul(
                                    pe_flat[:, 129:193], lhs(oa, icc, 2 * al, 2 * be),
                                    xts[icc][:, 65 - al:66 - al, 1 - be:65 - be],
                                    start=(j == 0), stop=(j == 15),
                                    skip_group_check=True)
                                j += 1
                    # R_o: zz[128, 2m+1] m=0..63 -> [193:257]
                    j = 0
                    for icc in range(N_ICC):
                        for al in (0, 1):
                            nc.tensor.matmul(
                                pe_flat[:, 193:257], lhs(oa, icc, 2 * al, 1),
                                xts[icc][:, 65 - al:66 - al, 1:65],
                                start=(j == 0), stop=(j == 7),
                                skip_group_check=True)
                            j += 1
                    # evac edges: zz[r,c] -> zzV[i=(r%2==0? via slot r+1...)]
                    # zz row 2k -> slot 2k+1 (i=1), row 2k+1 -> slot 2k+2 (i=0)
                    # zz col 128 -> slot 129 (j=1,c=64); col 2m -> slot 2m+1 (j=1)
                    nc.scalar.activation(
                        zzV[:, 1, 1, 0:65, 64:65],
                        pe_flat[:, 0:65].rearrange("p (r c) -> p r c", c=1), Copy)
                    nc.scalar.activation(
                        zzV[:, 0, 1, 1:65, 64:65],
                        pe_flat[:, 65:129].rearrange("p (r c) -> p r c", c=1), Copy)
                    nc.scalar.activation(
                        zzV[:, 1, 1, 64:65, 0:64],
                        pe_flat[:, 129:193].rearrange("p (r c) -> p r c", r=1), Copy)
                    nc.scalar.activation(
                        zzV[:, 1, 0, 64:65, 1:65],
                        pe_flat[:, 193:257].rearrange("p (r c) -> p r c", r=1), Copy)

                def pe_block(Kb):
                    r0 = 8 * Kb
                    # icc-outer emission: each x chunk unlocks 9 matmuls
                    # across all four parity groups, so the PE FIFO never
                    # stalls on a later icc DMA while earlier work exists
                    ps_ee = ppool.tile([128, 8, 64], f32, tag="pee")
                    ps_eo = ppool.tile([128, 8, 64], f32, tag="peo")
                    ps_oe = ppool.tile([128, 8, 64], f32, tag="poe")
                    ps_oo = ppool.tile([128, 8, 64], f32, tag="poo")
                    for icc in range(N_ICC):
                        first = icc == 0
                        last = icc == N_ICC - 1
                        je = 0
                        for al in (0, 1):
                            for be in (0, 1):
                                nc.tensor.matmul(
                                    ps_ee[:], lhs(oa, icc, 2 * al, 2 * be),
                                    xts[icc][:, r0 + 1 - al:r0 + 9 - al,
                                             1 - be:65 - be],
                                    start=(first and je == 0),
                                    stop=(last and je == 3))
                                je += 1
                        for al in (0, 1):
                            nc.tensor.matmul(
                                ps_eo[:], lhs(oa, icc, 2 * al, 1),
                                xts[icc][:, r0 + 1 - al:r0 + 9 - al, 1:65],
                                start=(first and al == 0),
                                stop=(last and al == 1))
                        for be in (0, 1):
                            nc.tensor.matmul(
                                ps_oe[:], lhs(oa, icc, 1, 2 * be),
                                xts[icc][:, r0 + 1:r0 + 9, 1 - be:65 - be],
                                start=(first and be == 0),
                                stop=(last and be == 1))
                        nc.tensor.matmul(
                            ps_oo[:], lhs(oa, icc, 1, 1),
                            xts[icc][:, r0 + 1:r0 + 9, 1:65],
                            start=first, stop=last)
                    # evac: row 16Kb+2k -> slot ..+1 (i=1, r=8Kb+k);
                    #       row 16Kb+2k+1 -> slot ..+2 (i=0, r=8Kb+k+1)
                    # col 2m -> slot 2m+1 (j=1, c=m); col 2m+1 -> slot 2m+2 (j=0, c=m+1)
                    if "evac" in _ABLATE:
                        return
                    nc.scalar.activation(zzV[:, 1, 1, r0:r0 + 8, 0:64], ps_ee[:], Copy)
                    nc.scalar.activation(zzV[:, 1, 0, r0:r0 + 8, 1:65], ps_eo[:], Copy)
                    nc.scalar.activation(zzV[:, 0, 1, r0 + 1:r0 + 9, 0:64], ps_oe[:], Copy)
                    nc.scalar.activation(zzV[:, 0, 0, r0 + 1:r0 + 9, 1:65], ps_oo[:], Copy)

                ytiles = {}

                def vh_stages(Kb):
                    # v1 window: rows [16K-2, 16K+14) of domain [-1, 129)
                    a0, a1 = max(16 * Kb - 2, -1), min(16 * Kb + 14, 129)
                    if a0 < a1:
                        nc.vector.tensor_tensor(
                            v1P[:, a0 + 1:a1 + 1, :],
                            zzP[:, a0 + 1:a1 + 1, :],
                            zzP[:, a0 + 2:a1 + 2, :], Add)
                    # v2 window: rows [16K-4, 16K+12) of domain [-1, 128)
                    b0, b1 = max(16 * Kb - 4, -1), min(16 * Kb + 12, 128)
                    r = b0
                    while r < b1:
                        s = (r + 1) % RING
                        n = min(b1 - r, RING - s)
                        nc.vector.tensor_tensor(
                            v2R[:, s:s + n, :],
                            v1P[:, r + 1:r + 1 + n, :],
                            v1P[:, r + 2:r + 2 + n, :], Add)
                        r += n
                    # out-rows window: [16K-6, 16K+10) of [0, 128)
                    c0, c1 = max(16 * Kb - 6, 0), min(16 * Kb + 10, 128)
                    if c0 >= c1:
                        return
                    nrows = c1 - c0
                    v3 = bpool.tile([128, 16, PW], bf16, tag="v3")
                    # v3[i] = v2[c0+i-1] + v2[c0+i] ; src slots (c0+i)%RING, (c0+i+1)%RING
                    r = c0
                    while r < c1:
                        s0 = r % RING
                        s1 = (r + 1) % RING
                        n = min(c1 - r, RING - s0, RING - s1)
                        nc.vector.tensor_tensor(
                            v3[:, r - c0:r - c0 + n, :],
                            v2R[:, s0:s0 + n, :],
                            v2R[:, s1:s1 + n, :], Add)
                        r += n
                    # h chain split by rows: DVE rows [0:nd), gpsimd rows [nd:nrows)
                    # (h passes are row-independent: no cross-engine waits)
                    h1 = bpool.tile([128, 16, PW], bf16, tag="h1")
                    h2 = bpool.tile([128, 16, PW], bf16, tag="h2")
                    y = bpool.tile([128, 16, 128], bf16, tag="y", bufs=3)
                    nd = min(int(round(H_DVE_FRAC * 16)), nrows)
                    if nd > 0:
                        nc.vector.tensor_tensor(
                            h1[:, 0:nd, 0:130],
                            v3[:, 0:nd, 0:130], v3[:, 0:nd, 1:131], Add)
                        nc.vector.tensor_tensor(
                            h2[:, 0:nd, 0:129],
                            h1[:, 0:nd, 0:129], h1[:, 0:nd, 1:130], Add)
                        nc.vector.tensor_tensor(
                            y[:, 0:nd, :],
                            h2[:, 0:nd, 0:128], h2[:, 0:nd, 1:129], Add)
                    if nd < nrows:
                        nc.gpsimd.tensor_tensor(
                            h1[:, nd:nrows, 0:130],
                            v3[:, nd:nrows, 0:130], v3[:, nd:nrows, 1:131], Add)
                        nc.gpsimd.tensor_tensor(
                            h2[:, nd:nrows, 0:129],
                            h1[:, nd:nrows, 0:129], h1[:, nd:nrows, 1:130], Add)
                        nc.gpsimd.tensor_tensor(
                            y[:, nd:nrows, :],
                            h2[:, nd:nrows, 0:128], h2[:, nd:nrows, 1:129], Add)
                    ytiles[Kb] = (y, c0, c1)

                def finish(Kb):
                    if Kb not in ytiles:
                        return
                    y, c0, c1 = ytiles.pop(Kb)
                    nrows = c1 - c0
                    if "prelu" not in _ABLATE:
                        func = (mybir.ActivationFunctionType.Identity
                                if "identity" in _ABLATE else Prelu)
                        nc.scalar.activation(
                            y[:, 0:nrows, :], y[:, 0:nrows, :], func,
                            bias=bg[:, oa:oa + 1],
                            scale=ACT_GAIN / 16.0, alpha=ALPHA)
                    # out stores ride the ACT HWDGE ring so x prefetch on the
                    # SP ring is never queued behind them
                    if "outdma" not in _ABLATE:
                        nc.scalar.dma_start(
                            out=out_ext[out_img, oa * 128:(oa + 1) * 128, c0:c1, :],
                            in_=y[:, 0:nrows, :])

                skip_stages = "stages" in _ABLATE
                skip_finish = "finish" in _ABLATE or skip_stages
                for Kb in range(N_B):
                    pe_block(Kb)
                    if Kb == 1:
                        edge_pass()
                    if not skip_stages and Kb >= 1:
                        vh_stages(Kb - 1)
                    if not skip_finish and Kb >= 2:
                        finish(Kb - 2)
                if not skip_stages:
                    vh_stages(N_B - 1)
                    vh_stages(N_B)
                if not skip_finish:
                    finish(N_B - 2)
                    finish(N_B - 1)
                    finish(N_B)

            for it in range(n_img * n_rep):
                img = it % n_img
                xts = []
                for icc in range(N_ICC):
                    xt = xpool.tile([128, H + 2, W + 2], bf16, tag=f"x{icc}")
                    nc.sync.dma_start(out=xt[:], in_=xq_ext[img, icc])
                    xts.append(xt)
                for oa in range(N_OCC):
                    unit(img, oa, xts, img)
    nc.compile()
    return nc


def _get_nc(n_img: int = IMG_PER_CORE, n_rep: int = 1):
    key = (n_img, n_rep)
    if key not in _CACHE:
        _CACHE[key] = _build_nc(n_img, n_rep)
    return _CACHE[key]


def kernel(x: np.ndarray, weight: np.ndarray, bias: np.ndarray) -> np.ndarray:
    from concourse.bass_utils import run_bass_kernel_spmd

    x = np.asarray(x, np.float32)
    weight = np.asarray(weight, np.float32)
    bias = np.asarray(bias, np.float32)

    xpad, wt, bg = _prep_inputs(x, weight, bias)

    nc = _get_nc()
    in_maps = []
    for c in range(N_CORES):
        sl = np.ascontiguousarray(xpad[c * IMG_PER_CORE:(c + 1) * IMG_PER_CORE])
        in_maps.append({"xq": sl, "wt": wt, "bg": bg})
    res = run_bass_kernel_spmd(nc, in_maps, list(range(N_CORES)))
    out = np.concatenate([res.results[c]["out"] for c in range(N_CORES)], axis=0)
    out = out.astype(np.float32)
    np.clip(out, -CLAMP, CLAMP, out=out)
    return out



# revision 16
# speedup vs baseline: 1.2075x; 1.2075x over previous
"""Trainium2 Bass kernel for StyleGAN2-style upsampling ConvLayer.

Reference computation (per image):
  zz = conv_transpose2d(x, (w*WSCALE), stride=2)      # 512ch 64x64 -> 256ch 129x129
  y  = upfirdn2d(zz, fir([1,3,3,1]), pad=1, gain=4)   # 4x4 blur   -> 128x128
  y  = clamp(lrelu(y + bias, 0.2) * sqrt(2), +-256)

This implementation keeps the FIR *out* of the tensor engine (the previous
version folded the horizontal FIR into the weights, doubling PE work):

  - PE computes the bare polyphase conv_transpose zz (129x129) in bf16:
    4 parity groups per 16-row block, psum tiles [128, 8, 64], N=512
    matmuls at 1 cyc/row.  Edge strips (row 128 / col 128) are packed
    into one extra psum pass per unit.
  - ACT evacuates PSUM -> SBUF with bf16 downcast and column
    de-interleave into a padded zz plane (row pitch 132 keeps every
    row-shifted view 4B-aligned).
  - DVE applies the vertical FIR [1,3,3,1] as three box passes in bf16
    2x mode, then h1 and part of h3 at 1x.
  - GPSIMD does h2 and the rest of h3.
  - ACT applies Prelu with the FIR normalization (1/16) and lrelu gain
    folded into scale/bias.  Output is bf16; the host upcasts to fp32
    and applies the (numerically inert) +-256 clamp exactly.

Sharding: data parallel, 2 images per core across 8 NeuronCores.
"""

import numpy as np
import ml_dtypes

N_CORES = 8
IMG_PER_CORE = 2
IN_CH, OUT_CH, K, UP = 512, 256, 3, 2
H = W = 64
WSCALE = float(1.0 / np.sqrt(K * K * IN_CH))
ACT_GAIN = float(np.sqrt(2.0))
CLAMP = 256.0
ALPHA = 0.2
N_ICC = IN_CH // 128   # 4 ic chunks
N_OCC = OUT_CH // 128  # 2 oc chunks
N_B = 8                # main blocks of 16 zz rows (rows 0..127)
PW = 132               # padded row width (even -> 264B pitch, keeps bf16 2x)
RING = 32              # v2 ring rows
H_DVE_FRAC = 11.0 / 16.0  # fraction of h-chain rows on DVE (rest gpsimd)
X_CHUNKS = (0, 18, 34, 50, 66)  # x DMA row chunks (sub-range deps)

_CACHE = {}
_ABLATE = set()  # debug: subsets of {"stages", "evac", "finish", "edges"}


def _prep_inputs(x, weight, bias):
    bf = ml_dtypes.bfloat16
    n = x.shape[0]
    xq = x.reshape(n, N_ICC, 128, H, W)
    xpad = np.zeros((n, N_ICC, 128, H + 2, W + 2), bf)
    xpad[:, :, :, 1:H + 1, 1:W + 1] = xq.astype(bf)
    # wt[i, (oa, icc, a, b), o]
    w = (weight.astype(np.float64) * WSCALE).astype(np.float32)
    w = w.reshape(N_OCC, 128, N_ICC, 128, K, K)          # [oa, o, icc, i, a, b]
    wt = np.ascontiguousarray(
        w.transpose(3, 0, 2, 4, 5, 1).astype(bf)         # [i, oa, icc, a, b, o]
    ).reshape(128, N_OCC * N_ICC * K * K * 128)
    bg = np.ascontiguousarray(
        (bias.astype(np.float64) * ACT_GAIN).astype(np.float32)
        .reshape(N_OCC, 128).T)
    return xpad, wt, bg


def _build_nc(n_img: int = IMG_PER_CORE, n_rep: int = 1):
    import concourse.bacc as bacc
    import concourse.mybir as mybir
    import concourse.tile as tile

    f32 = mybir.dt.float32
    bf16 = mybir.dt.bfloat16
    Prelu = mybir.ActivationFunctionType.Prelu
    Copy = mybir.ActivationFunctionType.Copy
    Add = mybir.AluOpType.add

    nc = bacc.Bacc()
    xq_ext = nc.declare_dram_parameter(
        "xq", [n_img, N_ICC, 128, H + 2, W + 2], bf16, isOutput=False)
    wt_ext = nc.declare_dram_parameter(
        "wt", [128, N_OCC * N_ICC * K * K * 128], bf16, isOutput=False)
    bg_ext = nc.declare_dram_parameter("bg", [128, N_OCC], f32, isOutput=False)
    out_ext = nc.declare_dram_parameter(
        "out", [n_img, OUT_CH, 2 * H, 2 * W], bf16, isOutput=True)

    def widx(oa, icc, a, b):
        return ((oa * N_ICC + icc) * K + a) * K + b

    with tile.TileContext(nc) as tc:
        with (
            tc.tile_pool(name="cpool", bufs=1) as cpool,
            tc.tile_pool(name="xpool", bufs=2) as xpool,
            tc.tile_pool(name="plane", bufs=1) as plane,
            tc.tile_pool(name="bpool", bufs=2) as bpool,
            tc.tile_pool(name="ppool", bufs=2, space="PSUM") as ppool,
        ):
            # weights ride the ACT ring in parallel with x loads on SP;
            # split per (oa, icc) so the first ldweights gates on 288KB.
            # bg rides last (first prelu is ~10us in)
            wt = cpool.tile([128, N_OCC * N_ICC * K * K * 128], bf16)
            csz = K * K * 128
            for oa_ in range(N_OCC):
                for icc_ in range(N_ICC):
                    i0 = (oa_ * N_ICC + icc_) * csz
                    nc.scalar.dma_start(out=wt[:, i0:i0 + csz],
                                        in_=wt_ext[:, i0:i0 + csz])
            bg = cpool.tile([128, N_OCC], f32)

            # persistent planes; sub-range deps give cross-unit pipelining
            zzP = plane.tile([128, PW, PW], bf16)    # zz row r -> slot r+1
            v1P = plane.tile([128, 130, PW], bf16)   # v1[r]=zz[r]+zz[r+1], slot r+1
            v2R = plane.tile([128, RING, PW], bf16)  # v2[r]=v1[r]+v1[r+1], slot (r+1)%RING
            # interleaved view of zzP: [p, rowpar, colpar, r, c]
            # row slot 2r+i, col slot 2c+j
            zzV = zzP[:].rearrange("p (r i) (c j) -> p i j r c", i=2, j=2)

            # pad memsets once per kernel: pads are never overwritten (evacs
            # only touch the interior), and per-unit memsets created false
            # DVE deps on the previous unit's trailing v1 reads
            nc.vector.memset(zzP[:, 0:1, :], 0.0)
            nc.vector.memset(zzP[:, 130:132, :], 0.0)
            nc.vector.memset(zzP[:, 1:130, 0:1], 0.0)
            nc.vector.memset(zzP[:, 1:130, 130:132], 0.0)

            def lhs(oa, icc, a, b):
                i = widx(oa, icc, a, b)
                return wt[:, i * 128:(i + 1) * 128]

            def unit(img, oa, xts, out_img):
                # ---- edge strips: col 128 (all rows), row 128 (cols 0..127) ----
                # scheduled after block 1 so startup PE work streams in
                # per-icc instead of stalling on the full x load
                def edge_pass():
                    pse = ppool.tile([128, 8, 64], f32, tag="pee")
                    pe_flat = pse[:].rearrange("p r c -> p (r c)")
                    # strip_e: zz[2k,128] k=0..64 -> [0:65]
                    j = 0
                    for icc in range(N_ICC):
                        for al in (0, 1):
                            for be in (0, 1):
                                nc.tensor.matmul(
                                    pe_flat[:, 0:65], lhs(oa, icc, 2 * al, 2 * be),
                                    xts[icc][:, 1 - al:66 - al, 65 - be:66 - be],
                                    start=(j == 0), stop=(j == 15),
                                    skip_group_check=True)
                                j += 1
                    # strip_o: zz[2k+1,128] k=0..63 -> [65:129]
                    j = 0
                    for icc in range(N_ICC):
                        for be in (0, 1):
                            nc.tensor.matmul(
                                pe_flat[:, 65:129], lhs(oa, icc, 1, 2 * be),
                                xts[icc][:, 1:65, 65 - be:66 - be],
                                start=(j == 0), stop=(j == 7),
                                skip_group_check=True)
                            j += 1
                    # R_e: zz[128, 2m] m=0..63 -> [129:193]
                    j = 0
                    for icc in range(N_ICC):
                        for al in (0, 1):
                            for be in (0, 1):
                                nc.tensor.matmul(
                                    pe_flat[:, 129:193], lhs(oa, icc, 2 * al, 2 * be),
                                    xts[icc][:, 65 - al:66 - al, 1 - be:65 - be],
                                    start=(j == 0), stop=(j == 15),
                                    skip_group_check=True)
                                j += 1
                    # R_o: zz[128, 2m+1] m=0..63 -> [193:257]
                    j = 0
                    for icc in range(N_ICC):
                        for al in (0, 1):
                            nc.tensor.matmul(
                                pe_flat[:, 193:257], lhs(oa, icc, 2 * al, 1),
                                xts[icc][:, 65 - al:66 - al, 1:65],
                                start=(j == 0), stop=(j == 7),
                                skip_group_check=True)
                            j += 1
                    # evac edges: zz[r,c] -> zzV[i=(r%2==0? via slot r+1...)]
                    # zz row 2k -> slot 2k+1 (i=1), row 2k+1 -> slot 2k+2 (i=0)
                    # zz col 128 -> slot 129 (j=1,c=64); col 2m -> slot 2m+1 (j=1)
                    nc.scalar.activation(
                        zzV[:, 1, 1, 0:65, 64:65],
                        pe_flat[:, 0:65].rearrange("p (r c) -> p r c", c=1), Copy)
                    nc.scalar.activation(
                        zzV[:, 0, 1, 1:65, 64:65],
                        pe_flat[:, 65:129].rearrange("p (r c) -> p r c", c=1), Copy)
                    nc.scalar.activation(
                        zzV[:, 1, 1, 64:65, 0:64],
                        pe_flat[:, 129:193].rearrange("p (r c) -> p r c", r=1), Copy)
                    nc.scalar.activation(
                        zzV[:, 1, 0, 64:65, 1:65],
                        pe_flat[:, 193:257].rearrange("p (r c) -> p r c", r=1), Copy)

                def pe_block(Kb):
                    r0 = 8 * Kb
                    # icc-outer emission: each x chunk unlocks 9 matmuls
                    # across all four parity groups, so the PE FIFO never
                    # stalls on a later icc DMA while earlier work exists
                    ps_ee = ppool.tile([128, 8, 64], f32, tag="pee")
                    ps_eo = ppool.tile([128, 8, 64], f32, tag="peo")
                    ps_oe = ppool.tile([128, 8, 64], f32, tag="poe")
                    ps_oo = ppool.tile([128, 8, 64], f32, tag="poo")
                    for icc in range(N_ICC):
                        first = icc == 0
                        last = icc == N_ICC - 1
                        je = 0
                        for al in (0, 1):
                            for be in (0, 1):
                                nc.tensor.matmul(
                                    ps_ee[:], lhs(oa, icc, 2 * al, 2 * be),
                                    xts[icc][:, r0 + 1 - al:r0 + 9 - al,
                                             1 - be:65 - be],
                                    start=(first and je == 0),
                                    stop=(last and je == 3))
                                je += 1
                        for al in (0, 1):
                            nc.tensor.matmul(
                                ps_eo[:], lhs(oa, icc, 2 * al, 1),
                                xts[icc][:, r0 + 1 - al:r0 + 9 - al, 1:65],
                                start=(first and al == 0),
                                stop=(last and al == 1))
                        for be in (0, 1):
                            nc.tensor.matmul(
                                ps_oe[:], lhs(oa, icc, 1, 2 * be),
                                xts[icc][:, r0 + 1:r0 + 9, 1 - be:65 - be],
                                start=(first and be == 0),
                                stop=(last and be == 1))
                        nc.tensor.matmul(
                            ps_oo[:], lhs(oa, icc, 1, 1),
                            xts[icc][:, r0 + 1:r0 + 9, 1:65],
                            start=first, stop=last)
                    # evac: row 16Kb+2k -> slot ..+1 (i=1, r=8Kb+k);
                    #       row 16Kb+2k+1 -> slot ..+2 (i=0, r=8Kb+k+1)
                    # col 2m -> slot 2m+1 (j=1, c=m); col 2m+1 -> slot 2m+2 (j=0, c=m+1)
                    if "evac" in _ABLATE:
                        return
                    nc.scalar.activation(zzV[:, 1, 1, r0:r0 + 8, 0:64], ps_ee[:], Copy)
                    nc.scalar.activation(zzV[:, 1, 0, r0:r0 + 8, 1:65], ps_eo[:], Copy)
                    nc.scalar.activation(zzV[:, 0, 1, r0 + 1:r0 + 9, 0:64], ps_oe[:], Copy)
                    nc.scalar.activation(zzV[:, 0, 0, r0 + 1:r0 + 9, 1:65], ps_oo[:], Copy)

                ytiles = {}

                def vh_win(key, a0, a1, b0, b1, c0, c1, col_split=False):
                    # v-pass engines: DVE only, or column-split DVE|gpsimd
                    # (columns are independent through the whole v chain)
                    if col_split:
                        cols = ((nc.vector, 0, 68), (nc.gpsimd, 68, PW))
                    else:
                        cols = ((nc.vector, 0, PW),)
                    # v1 rows [a0, a1) of domain [-1, 129)
                    if a0 < a1:
                        for eng, u0, u1 in cols:
                            eng.tensor_tensor(
                                v1P[:, a0 + 1:a1 + 1, u0:u1],
                                zzP[:, a0 + 1:a1 + 1, u0:u1],
                                zzP[:, a0 + 2:a1 + 2, u0:u1], Add)
                    # v2 rows [b0, b1) of domain [-1, 128)
                    r = b0
                    while r < b1:
                        s = (r + 1) % RING
                        n = min(b1 - r, RING - s)
                        for eng, u0, u1 in cols:
                            eng.tensor_tensor(
                                v2R[:, s:s + n, u0:u1],
                                v1P[:, r + 1:r + 1 + n, u0:u1],
                                v1P[:, r + 2:r + 2 + n, u0:u1], Add)
                        r += n
                    if c0 >= c1:
                        return
                    nrows = c1 - c0
                    v3 = bpool.tile([128, 16, PW], bf16, tag="v3",
                                    name=f"v3_{key}")
                    # v3[i] = v2[c0+i-1] + v2[c0+i]
                    r = c0
                    while r < c1:
                        s0 = r % RING
                        s1 = (r + 1) % RING
                        n = min(c1 - r, RING - s0, RING - s1)
                        for eng, u0, u1 in cols:
                            eng.tensor_tensor(
                                v3[:, r - c0:r - c0 + n, u0:u1],
                                v2R[:, s0:s0 + n, u0:u1],
                                v2R[:, s1:s1 + n, u0:u1], Add)
                        r += n
                    # h chain split by rows: DVE rows [0:nd), gpsimd rows [nd:nrows)
                    # (h passes are row-independent: no cross-engine waits)
                    h1 = bpool.tile([128, 16, PW], bf16, tag="h1",
                                    name=f"h1_{key}")
                    h2 = bpool.tile([128, 16, PW], bf16, tag="h2",
                                    name=f"h2_{key}")
                    y = bpool.tile([128, 16, 128], bf16, tag="y", bufs=3,
                                   name=f"y_{key}")
                    nd = min(int(round(H_DVE_FRAC * nrows)), nrows)
                    if nd > 0:
                        nc.vector.tensor_tensor(
                            h1[:, 0:nd, 0:130],
                            v3[:, 0:nd, 0:130], v3[:, 0:nd, 1:131], Add)
                        nc.vector.tensor_tensor(
                            h2[:, 0:nd, 0:129],
                            h1[:, 0:nd, 0:129], h1[:, 0:nd, 1:130], Add)
                        nc.vector.tensor_tensor(
                            y[:, 0:nd, :],
                            h2[:, 0:nd, 0:128], h2[:, 0:nd, 1:129], Add)
                    if nd < nrows:
                        nc.gpsimd.tensor_tensor(
                            h1[:, nd:nrows, 0:130],
                            v3[:, nd:nrows, 0:130], v3[:, nd:nrows, 1:131], Add)
                        nc.gpsimd.tensor_tensor(
                            h2[:, nd:nrows, 0:129],
                            h1[:, nd:nrows, 0:129], h1[:, nd:nrows, 1:130], Add)
                        nc.gpsimd.tensor_tensor(
                            y[:, nd:nrows, :],
                            h2[:, nd:nrows, 0:128], h2[:, nd:nrows, 1:129], Add)
                    ytiles[key] = (y, c0, c1, nd)

                def vh_stages(Kb):
                    # 16-row stage: v1 [16K-2,16K+14), v2 [16K-4,16K+12),
                    # out [16K-6,16K+10)
                    vh_win(f"s{Kb}",
                           max(16 * Kb - 2, -1), min(16 * Kb + 14, 129),
                           max(16 * Kb - 4, -1), min(16 * Kb + 12, 128),
                           max(16 * Kb - 6, 0), min(16 * Kb + 10, 128))

                def vh_half(m):
                    # 8-row tail stage (halved pipeline latency)
                    vh_win(f"h{m}",
                           max(8 * m - 2, -1), min(8 * m + 6, 129),
                           max(8 * m - 4, -1), min(8 * m + 4, 128),
                           max(8 * m - 6, 0), min(8 * m + 2, 128))

                def finish(key):
                    if key not in ytiles:
                        return
                    y, c0, c1, nd = ytiles.pop(key)
                    nrows = c1 - c0
                    func = (mybir.ActivationFunctionType.Identity
                            if "identity" in _ABLATE else Prelu)
                    # two halves matching the DVE/gpsimd h-split so each
                    # half's prelu+store flows as its producer finishes
                    for a, b in ((0, nd), (nd, nrows)):
                        if a >= b:
                            continue
                        if "prelu" not in _ABLATE:
                            nc.scalar.activation(
                                y[:, a:b, :], y[:, a:b, :], func,
                                bias=bg[:, oa:oa + 1],
                                scale=ACT_GAIN / 16.0, alpha=ALPHA)
                        # out stores ride the ACT HWDGE ring so x prefetch on
                        # the SP ring is never queued behind them
                        if "outdma" not in _ABLATE:
                            nc.scalar.dma_start(
                                out=out_ext[out_img, oa * 128:(oa + 1) * 128,
                                            c0 + a:c0 + b, :],
                                in_=y[:, a:b, :])

                skip_stages = "stages" in _ABLATE
                skip_finish = "finish" in _ABLATE or skip_stages
                for Kb in range(N_B):
                    pe_block(Kb)
                    if Kb == 1:
                        edge_pass()
                    if not skip_stages and Kb >= 1:
                        vh_stages(Kb - 1)
                    if not skip_finish and Kb >= 2:
                        finish(f"s{Kb - 2}")
                m0 = 2 * (N_B - 1)
                if not skip_stages:
                    for m in (m0, m0 + 1, m0 + 2):
                        vh_half(m)
                if not skip_finish:
                    # deprioritize trailing finishes so the scheduler breaks
                    # ready-ties in favor of the next unit's psum evacs
                    with tc.high_priority(offset=-200000):
                        finish(f"s{N_B - 2}")
                        for m in (m0, m0 + 1, m0 + 2):
                            finish(f"h{m}")

            first = True
            for it in range(n_img * n_rep):
                img = it % n_img
                xts = []
                for icc in range(N_ICC):
                    xt = xpool.tile([128, H + 2, W + 2], bf16, tag=f"x{icc}",
                                    name=f"xt{it}_{icc}")
                    xts.append(xt)
                # row-chunked, icc-interleaved loads: sub-range deps let the
                # first block's matmuls start after ~0.3MB instead of 4.4MB
                for j in range(len(X_CHUNKS) - 1):
                    r0, r1 = X_CHUNKS[j], X_CHUNKS[j + 1]
                    for icc in range(N_ICC):
                        nc.sync.dma_start(
                            out=xts[icc][:, r0:r1, :],
                            in_=xq_ext[img, icc, :, r0:r1, :])
                if first:
                    # bg load after the startup-critical x/wt chunks
                    nc.scalar.dma_start(out=bg[:], in_=bg_ext[:])
                    first = False
                for oa in range(N_OCC):
                    unit(img, oa, xts, img)
    nc.compile()
    return nc


def _get_nc(n_img: int = IMG_PER_CORE, n_rep: int = 1):
    key = (n_img, n_rep)
    if key not in _CACHE:
        _CACHE[key] = _build_nc(n_img, n_rep)
    return _CACHE[key]


def kernel(x: np.ndarray, weight: np.ndarray, bias: np.ndarray) -> np.ndarray:
    from concourse.bass_utils import run_bass_kernel_spmd

    x = np.asarray(x, np.float32)
    weight = np.asarray(weight, np.float32)
    bias = np.asarray(bias, np.float32)

    xpad, wt, bg = _prep_inputs(x, weight, bias)

    nc = _get_nc()
    in_maps = []
    for c in range(N_CORES):
        sl = np.ascontiguousarray(xpad[c * IMG_PER_CORE:(c + 1) * IMG_PER_CORE])
        in_maps.append({"xq": sl, "wt": wt, "bg": bg})
    res = run_bass_kernel_spmd(nc, in_maps, list(range(N_CORES)))
    out = np.concatenate([res.results[c]["out"] for c in range(N_CORES)], axis=0)
    out = out.astype(np.float32)
    np.clip(out, -CLAMP, CLAMP, out=out)
    return out



# revision 20
# speedup vs baseline: 1.2974x; 1.0744x over previous
"""Trainium2 Bass kernel for StyleGAN2-style upsampling ConvLayer.

Reference computation (per image):
  zz = conv_transpose2d(x, (w*WSCALE), stride=2)      # 512ch 64x64 -> 256ch 129x129
  y  = upfirdn2d(zz, fir([1,3,3,1]), pad=1, gain=4)   # 4x4 blur   -> 128x128
  y  = clamp(lrelu(y + bias, 0.2) * sqrt(2), +-256)

This implementation keeps the FIR *out* of the tensor engine (the previous
version folded the horizontal FIR into the weights, doubling PE work):

  - PE computes the bare polyphase conv_transpose zz (129x129) in bf16:
    4 parity groups per 16-row block, psum tiles [128, 8, 64], N=512
    matmuls at 1 cyc/row.  Edge strips (row 128 / col 128) are packed
    into one extra psum pass per unit.
  - ACT evacuates PSUM -> SBUF with bf16 downcast and column
    de-interleave into a padded zz plane (row pitch 132 keeps every
    row-shifted view 4B-aligned).
  - DVE applies the vertical FIR [1,3,3,1] as three box passes in bf16
    2x mode, then h1 and part of h3 at 1x.
  - GPSIMD does h2 and the rest of h3.
  - ACT applies Prelu with the FIR normalization (1/16) and lrelu gain
    folded into scale/bias.  Output is bf16; the host upcasts to fp32
    and applies the (numerically inert) +-256 clamp exactly.

Sharding: data parallel, 2 images per core across 8 NeuronCores.
"""

import numpy as np
import ml_dtypes

N_CORES = 8
IMG_PER_CORE = 2
IN_CH, OUT_CH, K, UP = 512, 256, 3, 2
H = W = 64
WSCALE = float(1.0 / np.sqrt(K * K * IN_CH))
ACT_GAIN = float(np.sqrt(2.0))
CLAMP = 256.0
ALPHA = 0.2
N_ICC = IN_CH // 128   # 4 ic chunks
N_OCC = OUT_CH // 128  # 2 oc chunks
N_B = 8                # main blocks of 16 zz rows (rows 0..127)
PW = 132               # padded row width (even -> 264B pitch, keeps bf16 2x)
RING = 32              # v2 ring rows
H_DVE_FRAC = 11.0 / 16.0  # fraction of h-chain rows on DVE (rest gpsimd)
X_CHUNKS = (0, 18, 34, 50, 66)  # x DMA row chunks (sub-range deps)

_CACHE = {}
_ABLATE = set()  # debug: subsets of {"stages", "evac", "finish", "edges"}


def _prep_inputs(x, weight, bias):
    bf = ml_dtypes.bfloat16
    n = x.shape[0]
    xq = x.reshape(n, N_ICC, 128, H, W)
    xpad = np.zeros((n, N_ICC, 128, H + 2, W + 2), bf)
    xpad[:, :, :, 1:H + 1, 1:W + 1] = xq.astype(bf)
    # wt[i, (oa, icc, a, b), o]
    w = (weight.astype(np.float64) * WSCALE).astype(np.float32)
    w = w.reshape(N_OCC, 128, N_ICC, 128, K, K)          # [oa, o, icc, i, a, b]
    wt = np.ascontiguousarray(
        w.transpose(3, 0, 2, 4, 5, 1).astype(bf)         # [i, oa, icc, a, b, o]
    ).reshape(128, N_OCC * N_ICC * K * K * 128)
    bg = np.ascontiguousarray(
        (bias.astype(np.float64) * ACT_GAIN).astype(np.float32)
        .reshape(N_OCC, 128).T)
    return xpad, wt, bg


def _build_nc(n_img: int = IMG_PER_CORE, n_rep: int = 1):
    import concourse.bacc as bacc
    import concourse.mybir as mybir
    import concourse.tile as tile

    f32 = mybir.dt.float32
    bf16 = mybir.dt.bfloat16
    Prelu = mybir.ActivationFunctionType.Prelu
    Copy = mybir.ActivationFunctionType.Copy
    Add = mybir.AluOpType.add

    nc = bacc.Bacc()
    xq_ext = nc.declare_dram_parameter(
        "xq", [n_img, N_ICC, 128, H + 2, W + 2], bf16, isOutput=False)
    wt_ext = nc.declare_dram_parameter(
        "wt", [128, N_OCC * N_ICC * K * K * 128], bf16, isOutput=False)
    bg_ext = nc.declare_dram_parameter("bg", [128, N_OCC], f32, isOutput=False)
    out_ext = nc.declare_dram_parameter(
        "out", [n_img, OUT_CH, 2 * H, 2 * W], bf16, isOutput=True)

    def widx(oa, icc, a, b):
        return ((oa * N_ICC + icc) * K + a) * K + b

    with tile.TileContext(nc) as tc:
        with (
            tc.tile_pool(name="cpool", bufs=1) as cpool,
            tc.tile_pool(name="xpool", bufs=2) as xpool,
            tc.tile_pool(name="plane", bufs=1) as plane,
            tc.tile_pool(name="bpool", bufs=2) as bpool,
            tc.tile_pool(name="ppool", bufs=2, space="PSUM") as ppool,
        ):
            # weights ride the ACT ring in parallel with x loads on SP;
            # split per (oa, icc) so the first ldweights gates on 288KB.
            # The very first weight tile (32KB) rides SP ahead of everything
            # so the first matmul is gated on ~0.4us of DMA.
            # bg rides last (first prelu is ~10us in)
            wt = cpool.tile([128, N_OCC * N_ICC * K * K * 128], bf16)
            csz = K * K * 128
            nc.sync.dma_start(out=wt[:, 0:128], in_=wt_ext[:, 0:128])
            for oa_ in range(N_OCC):
                for icc_ in range(N_ICC):
                    i0 = (oa_ * N_ICC + icc_) * csz
                    j0 = max(i0, 128)
                    nc.scalar.dma_start(out=wt[:, j0:i0 + csz],
                                        in_=wt_ext[:, j0:i0 + csz])
            bg = cpool.tile([128, N_OCC], f32)

            # persistent planes; sub-range deps give cross-unit pipelining
            zzP = plane.tile([128, PW, PW], bf16)    # zz row r -> slot r+1
            v1P = plane.tile([128, 130, PW], bf16)   # v1[r]=zz[r]+zz[r+1], slot r+1
            v2R = plane.tile([128, RING, PW], bf16)  # v2[r]=v1[r]+v1[r+1], slot (r+1)%RING
            # interleaved view of zzP: [p, rowpar, colpar, r, c]
            # row slot 2r+i, col slot 2c+j
            zzV = zzP[:].rearrange("p (r i) (c j) -> p i j r c", i=2, j=2)

            # pad memsets once per kernel: pads are never overwritten (evacs
            # only touch the interior), and per-unit memsets created false
            # DVE deps on the previous unit's trailing v1 reads
            nc.vector.memset(zzP[:, 0:1, :], 0.0)
            nc.vector.memset(zzP[:, 130:132, :], 0.0)
            nc.vector.memset(zzP[:, 1:130, 0:1], 0.0)
            nc.vector.memset(zzP[:, 1:130, 130:132], 0.0)

            def lhs(oa, icc, a, b):
                i = widx(oa, icc, a, b)
                return wt[:, i * 128:(i + 1) * 128]

            def unit(img, oa, xts, out_img):
                # ---- edge strips: col 128 (all rows), row 128 (cols 0..127) ----
                # scheduled after block 1 so startup PE work streams in
                # per-icc instead of stalling on the full x load
                def edge_pass():
                    pse = ppool.tile([128, 8, 64], f32, tag="pee")
                    pe_flat = pse[:].rearrange("p r c -> p (r c)")
                    # strip_e: zz[2k,128] k=0..64 -> [0:65]
                    j = 0
                    for icc in range(N_ICC):
                        for al in (0, 1):
                            for be in (0, 1):
                                nc.tensor.matmul(
                                    pe_flat[:, 0:65], lhs(oa, icc, 2 * al, 2 * be),
                                    xts[icc][:, 1 - al:66 - al, 65 - be:66 - be],
                                    start=(j == 0), stop=(j == 15),
                                    skip_group_check=True)
                                j += 1
                    # strip_o: zz[2k+1,128] k=0..63 -> [65:129]
                    j = 0
                    for icc in range(N_ICC):
                        for be in (0, 1):
                            nc.tensor.matmul(
                                pe_flat[:, 65:129], lhs(oa, icc, 1, 2 * be),
                                xts[icc][:, 1:65, 65 - be:66 - be],
                                start=(j == 0), stop=(j == 7),
                                skip_group_check=True)
                            j += 1
                    # R_e: zz[128, 2m] m=0..63 -> [129:193]
                    j = 0
                    for icc in range(N_ICC):
                        for al in (0, 1):
                            for be in (0, 1):
                                nc.tensor.matmul(
                                    pe_flat[:, 129:193], lhs(oa, icc, 2 * al, 2 * be),
                                    xts[icc][:, 65 - al:66 - al, 1 - be:65 - be],
                                    start=(j == 0), stop=(j == 15),
                                    skip_group_check=True)
                                j += 1
                    # R_o: zz[128, 2m+1] m=0..63 -> [193:257]
                    j = 0
                    for icc in range(N_ICC):
                        for al in (0, 1):
                            nc.tensor.matmul(
                                pe_flat[:, 193:257], lhs(oa, icc, 2 * al, 1),
                                xts[icc][:, 65 - al:66 - al, 1:65],
                                start=(j == 0), stop=(j == 7),
                                skip_group_check=True)
                            j += 1
                    # evac edges: zz[r,c] -> zzV[i=(r%2==0? via slot r+1...)]
                    # zz row 2k -> slot 2k+1 (i=1), row 2k+1 -> slot 2k+2 (i=0)
                    # zz col 128 -> slot 129 (j=1,c=64); col 2m -> slot 2m+1 (j=1)
                    nc.scalar.activation(
                        zzV[:, 1, 1, 0:65, 64:65],
                        pe_flat[:, 0:65].rearrange("p (r c) -> p r c", c=1), Copy)
                    nc.scalar.activation(
                        zzV[:, 0, 1, 1:65, 64:65],
                        pe_flat[:, 65:129].rearrange("p (r c) -> p r c", c=1), Copy)
                    nc.scalar.activation(
                        zzV[:, 1, 1, 64:65, 0:64],
                        pe_flat[:, 129:193].rearrange("p (r c) -> p r c", r=1), Copy)
                    nc.scalar.activation(
                        zzV[:, 1, 0, 64:65, 1:65],
                        pe_flat[:, 193:257].rearrange("p (r c) -> p r c", r=1), Copy)

                def pe_rows(r0, nr):
                    # icc-outer emission: each x chunk unlocks 9 matmuls
                    # across all four parity groups, so the PE FIFO never
                    # stalls on a later icc DMA while earlier work exists
                    ps_ee = ppool.tile([128, nr, 64], f32, tag="pee",
                                       name=f"ps_ee_{r0}")
                    ps_eo = ppool.tile([128, nr, 64], f32, tag="peo",
                                       name=f"ps_eo_{r0}")
                    ps_oe = ppool.tile([128, nr, 64], f32, tag="poe",
                                       name=f"ps_oe_{r0}")
                    ps_oo = ppool.tile([128, nr, 64], f32, tag="poo",
                                       name=f"ps_oo_{r0}")
                    for icc in range(N_ICC):
                        first = icc == 0
                        last = icc == N_ICC - 1
                        je = 0
                        for al in (0, 1):
                            for be in (0, 1):
                                nc.tensor.matmul(
                                    ps_ee[:], lhs(oa, icc, 2 * al, 2 * be),
                                    xts[icc][:, r0 + 1 - al:r0 + 1 + nr - al,
                                             1 - be:65 - be],
                                    start=(first and je == 0),
                                    stop=(last and je == 3))
                                je += 1
                        for al in (0, 1):
                            nc.tensor.matmul(
                                ps_eo[:], lhs(oa, icc, 2 * al, 1),
                                xts[icc][:, r0 + 1 - al:r0 + 1 + nr - al, 1:65],
                                start=(first and al == 0),
                                stop=(last and al == 1))
                        for be in (0, 1):
                            nc.tensor.matmul(
                                ps_oe[:], lhs(oa, icc, 1, 2 * be),
                                xts[icc][:, r0 + 1:r0 + 1 + nr, 1 - be:65 - be],
                                start=(first and be == 0),
                                stop=(last and be == 1))
                        nc.tensor.matmul(
                            ps_oo[:], lhs(oa, icc, 1, 1),
                            xts[icc][:, r0 + 1:r0 + 1 + nr, 1:65],
                            start=first, stop=last)
                    # evac: row 16Kb+2k -> slot ..+1 (i=1, r=8Kb+k);
                    #       row 16Kb+2k+1 -> slot ..+2 (i=0, r=8Kb+k+1)
                    # col 2m -> slot 2m+1 (j=1, c=m); col 2m+1 -> slot 2m+2 (j=0, c=m+1)
                    if "evac" in _ABLATE:
                        return
                    nc.scalar.activation(zzV[:, 1, 1, r0:r0 + nr, 0:64], ps_ee[:], Copy)
                    nc.scalar.activation(zzV[:, 1, 0, r0:r0 + nr, 1:65], ps_eo[:], Copy)
                    nc.scalar.activation(zzV[:, 0, 1, r0 + 1:r0 + 1 + nr, 0:64], ps_oe[:], Copy)
                    nc.scalar.activation(zzV[:, 0, 0, r0 + 1:r0 + 1 + nr, 1:65], ps_oo[:], Copy)

                def pe_block(Kb):
                    pe_rows(8 * Kb, 8)

                ytiles = {}

                def vh_win(key, a0, a1, b0, b1, c0, c1, col_split=False):
                    # v-pass engines: DVE only, or column-split DVE|gpsimd
                    # (columns are independent through the whole v chain)
                    if col_split:
                        cols = ((nc.vector, 0, 68), (nc.gpsimd, 68, PW))
                    else:
                        cols = ((nc.vector, 0, PW),)
                    # v1 rows [a0, a1) of domain [-1, 129)
                    if a0 < a1:
                        for eng, u0, u1 in cols:
                            eng.tensor_tensor(
                                v1P[:, a0 + 1:a1 + 1, u0:u1],
                                zzP[:, a0 + 1:a1 + 1, u0:u1],
                                zzP[:, a0 + 2:a1 + 2, u0:u1], Add)
                    # v2 rows [b0, b1) of domain [-1, 128)
                    r = b0
                    while r < b1:
                        s = (r + 1) % RING
                        n = min(b1 - r, RING - s)
                        for eng, u0, u1 in cols:
                            eng.tensor_tensor(
                                v2R[:, s:s + n, u0:u1],
                                v1P[:, r + 1:r + 1 + n, u0:u1],
                                v1P[:, r + 2:r + 2 + n, u0:u1], Add)
                        r += n
                    if c0 >= c1:
                        return
                    nrows = c1 - c0
                    v3 = bpool.tile([128, 16, PW], bf16, tag="v3",
                                    name=f"v3_{key}")
                    # v3[i] = v2[c0+i-1] + v2[c0+i]
                    r = c0
                    while r < c1:
                        s0 = r % RING
                        s1 = (r + 1) % RING
                        n = min(c1 - r, RING - s0, RING - s1)
                        for eng, u0, u1 in cols:
                            eng.tensor_tensor(
                                v3[:, r - c0:r - c0 + n, u0:u1],
                                v2R[:, s0:s0 + n, u0:u1],
                                v2R[:, s1:s1 + n, u0:u1], Add)
                        r += n
                    # h chain split by rows: DVE rows [0:nd), gpsimd rows [nd:nrows)
                    # (h passes are row-independent: no cross-engine waits)
                    h1 = bpool.tile([128, 16, PW], bf16, tag="h1",
                                    name=f"h1_{key}")
                    h2 = bpool.tile([128, 16, PW], bf16, tag="h2",
                                    name=f"h2_{key}")
                    y = bpool.tile([128, 16, 128], bf16, tag="y", bufs=3,
                                   name=f"y_{key}")
                    nd = min(int(round(H_DVE_FRAC * nrows)), nrows)
                    if nd > 0:
                        nc.vector.tensor_tensor(
                            h1[:, 0:nd, 0:130],
                            v3[:, 0:nd, 0:130], v3[:, 0:nd, 1:131], Add)
                        nc.vector.tensor_tensor(
                            h2[:, 0:nd, 0:129],
                            h1[:, 0:nd, 0:129], h1[:, 0:nd, 1:130], Add)
                        nc.vector.tensor_tensor(
                            y[:, 0:nd, :],
                            h2[:, 0:nd, 0:128], h2[:, 0:nd, 1:129], Add)
                    if nd < nrows:
                        nc.gpsimd.tensor_tensor(
                            h1[:, nd:nrows, 0:130],
                            v3[:, nd:nrows, 0:130], v3[:, nd:nrows, 1:131], Add)
                        nc.gpsimd.tensor_tensor(
                            h2[:, nd:nrows, 0:129],
                            h1[:, nd:nrows, 0:129], h1[:, nd:nrows, 1:130], Add)
                        nc.gpsimd.tensor_tensor(
                            y[:, nd:nrows, :],
                            h2[:, nd:nrows, 0:128], h2[:, nd:nrows, 1:129], Add)
                    ytiles[key] = (y, c0, c1, nd)

                def vh_stages(Kb):
                    # 16-row stage: v1 [16K-2,16K+14), v2 [16K-4,16K+12),
                    # out [16K-6,16K+10)
                    vh_win(f"s{Kb}",
                           max(16 * Kb - 2, -1), min(16 * Kb + 14, 129),
                           max(16 * Kb - 4, -1), min(16 * Kb + 12, 128),
                           max(16 * Kb - 6, 0), min(16 * Kb + 10, 128))

                def vh_half(m):
                    # 8-row tail stage (halved pipeline latency)
                    vh_win(f"h{m}",
                           max(8 * m - 2, -1), min(8 * m + 6, 129),
                           max(8 * m - 4, -1), min(8 * m + 4, 128),
                           max(8 * m - 6, 0), min(8 * m + 2, 128))

                def finish(key):
                    if key not in ytiles:
                        return
                    y, c0, c1, nd = ytiles.pop(key)
                    nrows = c1 - c0
                    func = (mybir.ActivationFunctionType.Identity
                            if "identity" in _ABLATE else Prelu)
                    # two halves matching the DVE/gpsimd h-split so each
                    # half's prelu+store flows as its producer finishes
                    for a, b in ((0, nd), (nd, nrows)):
                        if a >= b:
                            continue
                        if "prelu" not in _ABLATE:
                            nc.scalar.activation(
                                y[:, a:b, :], y[:, a:b, :], func,
                                bias=bg[:, oa:oa + 1],
                                scale=ACT_GAIN / 16.0, alpha=ALPHA)
                        # out stores ride the ACT HWDGE ring so x prefetch on
                        # the SP ring is never queued behind them
                        if "outdma" not in _ABLATE:
                            nc.scalar.dma_start(
                                out=out_ext[out_img, oa * 128:(oa + 1) * 128,
                                            c0 + a:c0 + b, :],
                                in_=y[:, a:b, :])

                skip_stages = "stages" in _ABLATE
                skip_finish = "finish" in _ABLATE or skip_stages
                for Kb in range(N_B - 1):
                    pe_block(Kb)
                    if Kb == 1:
                        edge_pass()
                    if not skip_stages and Kb >= 1:
                        vh_stages(Kb - 1)
                    if not skip_finish and Kb >= 2:
                        finish(f"s{Kb - 2}")
                # last block as two 4-psum-row halves in separate banks, so
                # the first tail FIR stage overlaps the final matmul sweep
                m0 = 2 * (N_B - 1)
                pe_rows(8 * (N_B - 1), 4)
                if not skip_stages:
                    vh_stages(N_B - 2)
                if not skip_finish:
                    finish(f"s{N_B - 3}")
                pe_rows(8 * (N_B - 1) + 4, 4)
                if not skip_stages:
                    vh_half(m0)
                if not skip_finish:
                    finish(f"s{N_B - 2}")
                if not skip_stages:
                    vh_half(m0 + 1)
                    vh_half(m0 + 2)
                if not skip_finish:
                    # deprioritize trailing finishes so the scheduler breaks
                    # ready-ties in favor of the next unit's psum evacs
                    with tc.high_priority(offset=-200000):
                        for m in (m0, m0 + 1, m0 + 2):
                            finish(f"h{m}")

            first = True
            for it in range(n_img * n_rep):
                img = it % n_img
                xts = []
                for icc in range(N_ICC):
                    xt = xpool.tile([128, H + 2, W + 2], bf16, tag=f"x{icc}",
                                    name=f"xt{it}_{icc}")
                    xts.append(xt)
                # row-chunked, icc-interleaved loads: sub-range deps let the
                # first block's matmuls start after ~0.3MB instead of 4.4MB
                for j in range(len(X_CHUNKS) - 1):
                    r0, r1 = X_CHUNKS[j], X_CHUNKS[j + 1]
                    for icc in range(N_ICC):
                        nc.sync.dma_start(
                            out=xts[icc][:, r0:r1, :],
                            in_=xq_ext[img, icc, :, r0:r1, :])
                if first:
                    # bg load after the startup-critical x/wt chunks
                    nc.scalar.dma_start(out=bg[:], in_=bg_ext[:])
                    first = False
                for oa in range(N_OCC):
                    unit(img, oa, xts, img)
    nc.compile()
    return nc


def _get_nc(n_img: int = IMG_PER_CORE, n_rep: int = 1):
    key = (n_img, n_rep)
    if key not in _CACHE:
        _CACHE[key] = _build_nc(n_img, n_rep)
    return _CACHE[key]


def kernel(x: np.ndarray, weight: np.ndarray, bias: np.ndarray) -> np.ndarray:
    from concourse.bass_utils import run_bass_kernel_spmd

    x = np.asarray(x, np.float32)
    weight = np.asarray(weight, np.float32)
    bias = np.asarray(bias, np.float32)

    xpad, wt, bg = _prep_inputs(x, weight, bias)

    nc = _get_nc()
    in_maps = []
    for c in range(N_CORES):
        sl = np.ascontiguousarray(xpad[c * IMG_PER_CORE:(c + 1) * IMG_PER_CORE])
        in_maps.append({"xq": sl, "wt": wt, "bg": bg})
    try:
        res = run_bass_kernel_spmd(nc, in_maps, list(range(N_CORES)))
    except Exception:
        # transient device/transport errors have been observed; retry once
        res = run_bass_kernel_spmd(nc, in_maps, list(range(N_CORES)))
    out = np.concatenate([res.results[c]["out"] for c in range(N_CORES)], axis=0)
    out = out.astype(np.float32)
    np.clip(out, -CLAMP, CLAMP, out=out)
    return out

